# revision 11
# baseline (speedup 1.0000x reference)
"""CycleVAR VQ-codebook encoder kernel for Trainium2 (8 NeuronCores).

Contract: kernel(**inputs) takes FULL inputs
  f_src      [128, 32, 16, 16] fp32
  emb_weight [4096, 32] fp32
and returns the FULL output x_var [128, 340, 32] fp32.

x_var depends only on stages pn in (1, 2, 4, 8) (pn=16 stage is dead code);
the straight-through output equals the hard argmax embedding.

Sharding: data-parallel over batch (16 images per core), codebook replicated.

Reformulation (residual recursion pushed into tiny precomputed matrices):
  z_si  = A_si f  -  sum_{k<si} (A_si U_k) h_k
  x_si  = sum_{k<=si} (A_{si+1} U_k) h_k  (si<3);   x_3 = sum_k U_k h_k
so no f_rest / f_partial state lives on chip; h_k is the per-stage argmax
embedding in [p, (b,c)] layout (zero-padded rows so every PSUM accumulation
group uses a uniform contraction size).

Per 128-token block: scores = zaug^T eaug (K=33 fp32 matmuls, two PE
row-tiles, 4 PSUM waves of [t,1024]); ACT drains the chunk-1 waves to SBUF;
DVE merges chunk0(PSUM) vs chunk1(SBUF) with tensor_tensor(max), then
max8 + max_index on the merged row; the winning chunk is identified by a
1-element gpsimd indirect_copy of the chunk-1 value compared against the
max; vidx -> indirect DMA gather of the embedding row; per-image DMA
scatter into h layout.
"""

import os

import numpy as np

import concourse.bacc as bacc
import concourse.bass as bass
import concourse.mybir as mybir
import concourse.tile as tile
from concourse.bass import IndirectOffsetOnAxis
from concourse.bass_utils import run_bass_kernel_spmd

N_CORES = 8
B_FULL = 128
B_LOC = B_FULL // N_CORES  # 16
C = 32
H = 16
S = H * H  # 256
V = 4096
PNS = (1, 2, 4, 8)
ROW_OFF = (0, 4, 20, 84)
NTOK_OUT = 340

F32 = mybir.dt.float32
BF16 = mybir.dt.bfloat16
U16 = mybir.dt.uint16
U32 = mybir.dt.uint32
AX = mybir.AxisListType
ALU = mybir.AluOpType
ACTF = mybir.ActivationFunctionType

LAST_RESULTS = None


def _keys_cubic(x, a=-0.5):
    x = np.abs(x)
    return np.where(
        x <= 1,
        (a + 2) * x**3 - (a + 3) * x**2 + 1,
        np.where(x < 2, a * x**3 - 5 * a * x**2 + 8 * a * x - 4 * a, 0.0),
    )


def _resize_matrix_1d(n_in, n_out):
    scale = n_out / n_in
    U = np.zeros((n_out, n_in), np.float64)
    for i in range(n_out):
        x = (i + 0.5) / scale - 0.5
        w = _keys_cubic(x - np.arange(n_in))
        s = w.sum()
        if s != 0:
            w = w / s
        U[i] = w
    return U


def _up_matrix(pn):
    if pn == H:
        return np.eye(S)
    U1 = _resize_matrix_1d(pn, H)
    return np.kron(U1, U1)


def _down_matrix(pn):
    r = H // pn
    A = np.zeros((pn * pn, S), np.float64)
    w = 1.0 / (r * r)
    for pi in range(pn):
        for pj in range(pn):
            for di in range(r):
                for dj in range(r):
                    A[pi * pn + pj, (pi * r + di) * H + (pj * r + dj)] = w
    return A


def _to_bf16(a):
    import ml_dtypes

    return np.ascontiguousarray(np.asarray(a).astype(np.float32).astype(ml_dtypes.bfloat16))


def _build_program():
    nc = bacc.Bacc(trn_type="TRN2", target_bir_lowering=False, debug=False)

    eaug_in = nc.dram_tensor("eaug", [97, V], F32, kind="ExternalInput").ap()
    embB_in = nc.dram_tensor("embB", [V, 33], F32, kind="ExternalInput").ap()
    i33_in = nc.dram_tensor("i33", [33, 33], F32, kind="ExternalInput").ap()
    zaug0_in = nc.dram_tensor("zaug0", [97, 16], F32, kind="ExternalInput").ap()
    f_in = nc.dram_tensor("f_pre", [2, 128, 512], F32, kind="ExternalInput").ap()
    a_in = {
        si: nc.dram_tensor(
            f"a{si}", [2, 128, PNS[si] ** 2], F32, kind="ExternalInput"
        ).ap()
        for si in (1, 2, 3)
    }
    # negated correction mats, zero-padded to K=128: [128, P_si]
    mau_in = {
        (si, k): nc.dram_tensor(
            f"mau{si}_{k}", [128, PNS[si] ** 2], F32, kind="ExternalInput"
        ).ap()
        for si in (1, 2, 3)
        for k in range(si)
    }
    # x mats (bf16): gx[si][k] zero-padded to K=32: [32, P_{si+1}] for si<3
    gx_in = {
        (si, k): nc.dram_tensor(
            f"gx{si}_{k}", [32, PNS[si + 1] ** 2], BF16, kind="ExternalInput"
        ).ap()
        for si in range(3)
        for k in range(si + 1)
    }
    # x3 mats zero-padded to K=64: [64, 256]
    ux_in = {
        k: nc.dram_tensor(f"ux{k}", [64, 256], BF16, kind="ExternalInput").ap()
        for k in range(4)
    }
    x_out = nc.dram_tensor("xout", [NTOK_OUT, 512], F32, kind="ExternalOutput").ap()

    with tile.TileContext(nc) as tc:
        from contextlib import ExitStack

        ctx = ExitStack()
        const = ctx.enter_context(tc.tile_pool(name="const", bufs=1))
        work = ctx.enter_context(tc.tile_pool(name="work", bufs=2))
        small = ctx.enter_context(tc.tile_pool(name="small", bufs=3))
        psA = ctx.enter_context(tc.tile_pool(name="psA", bufs=1, space="PSUM"))
        psB = ctx.enter_context(tc.tile_pool(name="psB", bufs=1, space="PSUM"))
        psC = ctx.enter_context(tc.tile_pool(name="psC", bufs=2, space="PSUM"))

        # ---- constants ----
        eaug = const.tile([97, V], F32)
        nc.sync.dma_start(eaug[0:33], eaug_in[0:33])
        nc.scalar.dma_start(eaug[64:97], eaug_in[64:97])
        f_pre = [const.tile([128, 512], F32, name=f"fpre{ch}") for ch in range(2)]
        for ch in range(2):
            nc.sync.dma_start(f_pre[ch], f_in[ch])
        zaug = {0: const.tile([97, 16], F32, name="zaug0")}
        nc.scalar.dma_start(zaug[0], zaug0_in)
        i33 = const.tile([33, 33], F32, name="i33")
        nc.scalar.dma_start(i33, i33_in)
        for si in (1, 2, 3):
            P = PNS[si] ** 2
            zaug[si] = const.tile([97, 16 * P], F32, name=f"zaug{si}")
            nc.vector.memset(zaug[si][32:33], 1.0)
            nc.vector.memset(zaug[si][96:97], 1.0)
        a_sb = {}
        for si in (1, 2, 3):
            P = PNS[si] ** 2
            a_sb[si] = [const.tile([128, P], F32, name=f"a{si}_{ch}") for ch in range(2)]
            for ch in range(2):
                nc.sync.dma_start(a_sb[si][ch], a_in[si][ch])
        mau = {}
        for key, ap in mau_in.items():
            mau[key] = const.tile(list(ap.shape), F32, name=f"mau{key[0]}_{key[1]}")
            nc.scalar.dma_start(mau[key], ap)
        gx = {}
        for key, ap in gx_in.items():
            gx[key] = const.tile(list(ap.shape), BF16, name=f"gx{key[0]}_{key[1]}")
            nc.sync.dma_start(gx[key], ap)
        ux = {}
        for k, ap in ux_in.items():
            ux[k] = const.tile(list(ap.shape), BF16, name=f"ux{k}")
            nc.scalar.dma_start(ux[k], ap)

        # h state: fp32 padded to 128 rows (corrections, K=128);
        # bf16 padded to 64 rows (x matmuls, K=32/64)
        h_sp = {}
        h16 = {}
        for si in range(4):
            if si < 3:
                h_sp[si] = const.tile([128, 512], F32, name=f"h{si}")
                nc.vector.memset(h_sp[si], 0.0)
            else:
                h_sp[si] = const.tile([64, 512], F32, name="h3")
                nc.vector.memset(h_sp[si], 0.0)
            h16[si] = const.tile([64, 512], BF16, name=f"h16_{si}")
            nc.vector.memset(h16[si], 0.0)

        for si, pn in enumerate(PNS):
            P = pn * pn
            ntok = B_LOC * P
            nblk = (ntok + 127) // 128

            # ---- z = A f - corrections -> fold into zaug (si>0) ----
            if si > 0:
                for g in range(4):
                    psz = psB.tile([128, 64], F32, tag="psz")
                    gsl = slice(128 * g, 128 * (g + 1))
                    for ch in range(2):
                        nc.tensor.matmul(
                            psz[:, :P],
                            f_pre[ch][:, gsl],
                            a_sb[si][ch],
                            start=(ch == 0),
                            stop=False,
                        )
                    for k in range(si):
                        nc.tensor.matmul(
                            psz[:, :P],
                            h_sp[k][:, gsl],
                            mau[(si, k)],
                            start=False,
                            stop=(k == si - 1),
                        )
                    for j in range(4):
                        b = 4 * g + j
                        nc.scalar.activation(
                            zaug[si][0:32, b * P : (b + 1) * P],
                            psz[32 * j : 32 * (j + 1), :P],
                            ACTF.Copy,
                        )
                        nc.scalar.activation(
                            zaug[si][64:96, b * P : (b + 1) * P],
                            psz[32 * j : 32 * (j + 1), :P],
                            ACTF.Copy,
                        )

            # ---- token blocks ----
            for blk in range(nblk):
                t0 = blk * 128
                tw = min(128, ntok - t0)
                zb = zaug[si][:, t0 : t0 + tw]

                sb1 = work.tile([128, 2048], F32, tag="sb1")
                merged = work.tile([128, 2048], F32, tag="mrg")
                for h in range(2):
                    c0 = psA.tile([128, 1024], F32, tag="c0")
                    c1 = psA.tile([128, 1024], F32, tag="c1")
                    for q in range(2):
                        cs = slice(512 * q, 512 * (q + 1))
                        e0 = 1024 * h + 512 * q
                        nc.tensor.matmul(
                            c0[:tw, cs],
                            zb[0:33],
                            eaug[0:33, e0 : e0 + 512],
                            start=True,
                            stop=True,
                            tile_position=(0, 0),
                        )
                        nc.tensor.matmul(
                            c1[:tw, cs],
                            zb[64:97],
                            eaug[64:97, 2048 + e0 : 2048 + e0 + 512],
                            start=True,
                            stop=True,
                            tile_position=(64, 0),
                        )
                    hs = slice(1024 * h, 1024 * (h + 1))
                    nc.scalar.activation(sb1[:tw, hs], c1[:tw], ACTF.Copy)
                    nc.vector.tensor_tensor(
                        merged[:tw, hs], c0[:tw], sb1[:tw, hs], op=ALU.max
                    )

                top8 = small.tile([128, 8], F32, tag="top8")
                j8 = small.tile([128, 8], U32, tag="j8")
                nc.vector.max(top8[:tw], merged[:tw])
                nc.vector.max_index(j8[:tw], top8[:tw], merged[:tw])

                # zrow via PE transpose + ACT drain
                zrp = psC.tile([128, 33], F32, tag="zr")
                nc.tensor.transpose(zrp[:tw, :], zb[0:33], i33)
                zrow = small.tile([128, 33], F32, tag="zrow")
                nc.scalar.activation(zrow[:tw], zrp[:tw], ACTF.Copy)

                # candidate gathers + rescore + select embedding
                cand1 = small.tile([128, 1], U32, tag="cand1")
                nc.vector.tensor_scalar_add(cand1[:tw], j8[:tw, 0:1], 2048)
                e_a = small.tile([128, 33], F32, tag="ea")
                e_b = small.tile([128, 33], F32, tag="eb")
                nc.gpsimd.indirect_dma_start(
                    e_a[:tw], None, embB_in,
                    IndirectOffsetOnAxis(ap=j8[:tw, 0:1], axis=0),
                )
                nc.gpsimd.indirect_dma_start(
                    e_b[:tw], None, embB_in,
                    IndirectOffsetOnAxis(ap=cand1[:tw], axis=0),
                )
                ta = small.tile([128, 33], F32, tag="ta")
                tb = small.tile([128, 33], F32, tag="tb")
                nc.vector.tensor_tensor(ta[:tw], zrow[:tw], e_a[:tw], op=ALU.mult)
                nc.vector.tensor_tensor(tb[:tw], zrow[:tw], e_b[:tw], op=ALU.mult)
                sa = small.tile([128, 1], F32, tag="sa")
                sbv = small.tile([128, 1], F32, tag="sbv")
                nc.vector.reduce_sum(sa[:tw], ta[:tw], axis=AX.X)
                nc.vector.reduce_sum(sbv[:tw], tb[:tw], axis=AX.X)
                gef = small.tile([128, 1], F32, tag="gef")
                nc.vector.tensor_tensor(gef[:tw], sa[:tw], sbv[:tw], op=ALU.is_lt)
                dd = small.tile([128, C], F32, tag="dd")
                nc.vector.tensor_tensor(
                    dd[:tw], e_b[:tw, 0:C], e_a[:tw, 0:C], op=ALU.subtract
                )
                htok = small.tile([128, C], F32, tag="htok")
                nc.vector.scalar_tensor_tensor(
                    htok[:tw], in0=dd[:tw], scalar=gef[:tw], in1=e_a[:tw, 0:C],
                    op0=ALU.mult, op1=ALU.add,
                )
                # scatter into h_sp [p, (b,c)], one DMA per image
                qs = [nc.sync, nc.scalar, nc.gpsimd]
                for j, b in enumerate(range(t0 // P, (t0 + tw + P - 1) // P)):
                    r0 = b * P - t0
                    qs[j % 3].dma_start(
                        h_sp[si][0:P, 32 * b : 32 * (b + 1)],
                        htok[r0 : r0 + P],
                    )

            # ---- h16 convert ----
            nc.scalar.activation(h16[si][0:P], h_sp[si][0:P], ACTF.Copy)

            # ---- x output ----
            if si < 3:
                P2 = PNS[si + 1] ** 2
                px = psB.tile([128, 512], F32, tag="px")
                for k in range(si + 1):
                    nc.tensor.matmul(
                        px[:P2],
                        gx[(si, k)],
                        h16[k][0:32],
                        start=(k == 0),
                        stop=(k == si),
                    )
                x_sb = small.tile([128, 512], F32, tag="xsb")
                nc.scalar.activation(x_sb[:P2], px[:P2], ACTF.Copy)
                nc.sync.dma_start(x_out[ROW_OFF[si] : ROW_OFF[si] + P2], x_sb[:P2])
            else:
                for ch in range(2):
                    px = psB.tile([128, 512], F32, tag="px")
                    for k in range(4):
                        nc.tensor.matmul(
                            px,
                            ux[k][:, 128 * ch : 128 * (ch + 1)],
                            h16[k],
                            start=(k == 0),
                            stop=(k == 3),
                        )
                    x_sb = small.tile([128, 512], F32, tag="xsb")
                    nc.scalar.activation(x_sb, px, ACTF.Copy)
                    nc.sync.dma_start(
                        x_out[84 + 128 * ch : 84 + 128 * (ch + 1)], x_sb
                    )

        ctx.close()

    nc.compile()
    return nc


_PROGRAM = None


def _get_program():
    global _PROGRAM
    if _PROGRAM is None:
        _PROGRAM = _build_program()
    return _PROGRAM


def _host_prep(f_src, emb_weight):
    e64 = emb_weight.astype(np.float64)
    bias = -0.5 * (e64 * e64).sum(1)
    eaug = np.zeros((97, V), np.float64)
    eaug[0:32] = e64.T
    eaug[32] = bias
    eaug[64:96] = e64.T
    eaug[96] = bias
    eaug = eaug.astype(np.float32)
    embB = np.ascontiguousarray(
        np.concatenate([e64, bias[:, None]], axis=1).astype(np.float32)
    )  # [V, 33]

    A = {si: _down_matrix(pn) for si, pn in enumerate(PNS)}
    U = {si: _up_matrix(pn) for si, pn in enumerate(PNS)}

    a_mats = {
        si: np.ascontiguousarray(
            A[si].T.reshape(2, 128, PNS[si] ** 2).astype(np.float32)
        )
        for si in (1, 2, 3)
    }
    mau = {}
    for si in (1, 2, 3):
        for k in range(si):
            m = A[si] @ U[k]  # [P_si, P_k]
            pad = np.zeros((128, PNS[si] ** 2), np.float64)
            pad[0 : PNS[k] ** 2] = -m.T
            mau[(si, k)] = pad.astype(np.float32)
    gx = {}
    for si in range(3):
        for k in range(si + 1):
            m = A[si + 1] @ U[k]
            pad = np.zeros((32, PNS[si + 1] ** 2))
            pad[0 : PNS[k] ** 2] = m.T
            gx[(si, k)] = _to_bf16(pad)
    ux = {}
    for k in range(4):
        pad = np.zeros((64, 256))
        pad[0 : PNS[k] ** 2] = U[k].T
        ux[k] = _to_bf16(pad)

    f64 = f_src.astype(np.float64).reshape(B_FULL, C, S)

    in_maps = []
    for core in range(N_CORES):
        fb = f64[core * B_LOC : (core + 1) * B_LOC]  # [16, 32, 256]
        f_pre = np.ascontiguousarray(
            fb.transpose(2, 0, 1).reshape(2, 128, 512).astype(np.float32)
        )  # [s-chunk, s, (b,c)]
        z0 = fb.mean(axis=2)  # [16, 32]
        za0 = np.zeros((97, 16), np.float64)
        za0[0:32] = z0.T
        za0[32] = 1.0
        za0[64:96] = z0.T
        za0[96] = 1.0
        m = {
            "eaug": eaug,
            "embB": embB,
            "i33": np.eye(33, dtype=np.float32),
            "zaug0": za0.astype(np.float32),
            "f_pre": f_pre,
        }
        for si in (1, 2, 3):
            m[f"a{si}"] = a_mats[si]
        for (si, k), v in mau.items():
            m[f"mau{si}_{k}"] = v
        for (si, k), v in gx.items():
            m[f"gx{si}_{k}"] = v
        for k, v in ux.items():
            m[f"ux{k}"] = v
        in_maps.append(m)
    return in_maps


def kernel(f_src, emb_weight):
    global LAST_RESULTS
    f_src = np.asarray(f_src, dtype=np.float32)
    emb_weight = np.asarray(emb_weight, dtype=np.float32)

    in_maps = _host_prep(f_src, emb_weight)

    nc = _get_program()
    trace = bool(os.environ.get("CVAR_TRACE"))
    try:
        res = run_bass_kernel_spmd(
            nc, in_maps, core_ids=list(range(N_CORES)), trace=trace
        )
    except ModuleNotFoundError:
        res = run_bass_kernel_spmd(
            nc, in_maps, core_ids=list(range(N_CORES)), trace=False
        )
    LAST_RESULTS = res

    outs = []
    for core in range(N_CORES):
        xo = res.results[core]["xout"]  # [340, 512] in [p', (b,c)]
        outs.append(xo.reshape(NTOK_OUT, B_LOC, C).transpose(1, 0, 2))
    return np.ascontiguousarray(np.concatenate(outs, axis=0))


# revision 17
# speedup vs baseline: 1.0598x; 1.0598x over previous
"""CycleVAR VQ-codebook encoder kernel for Trainium2 (8 NeuronCores).

Contract: kernel(**inputs) takes FULL inputs
  f_src      [128, 32, 16, 16] fp32
  emb_weight [4096, 32] fp32
and returns the FULL output x_var [128, 340, 32] fp32.

x_var depends only on stages pn in (1, 2, 4, 8) (pn=16 stage is dead code);
the straight-through output equals the hard argmax embedding.

Sharding: data-parallel over batch (16 images per core), codebook replicated.

Reformulation (residual recursion pushed into tiny precomputed matrices):
  z_si  = A_si f  -  sum_{k<si} (A_si U_k) h_k
  x_si  = sum_{k<=si} (A_{si+1} U_k) h_k  (si<3);   x_3 = sum_k U_k h_k
so no f_rest / f_partial state lives on chip; h_k is the per-stage argmax
embedding in [p, (b,c)] layout (zero-padded rows so every PSUM accumulation
group uses a uniform contraction size).

Per 128-token block: scores = zaug^T eaug (K=33 fp32 matmuls, two PE
row-tiles, 4 PSUM waves of [t,1024]); ACT drains the chunk-1 waves to SBUF;
DVE merges chunk0(PSUM) vs chunk1(SBUF) with tensor_tensor(max), then
max8 + max_index on the merged row; the winning chunk is identified by a
1-element gpsimd indirect_copy of the chunk-1 value compared against the
max; vidx -> indirect DMA gather of the embedding row; per-image DMA
scatter into h layout.
"""

import os

import numpy as np

import concourse.bacc as bacc
import concourse.bass as bass
import concourse.mybir as mybir
import concourse.tile as tile
from concourse.bass import IndirectOffsetOnAxis
from concourse.bass_utils import run_bass_kernel_spmd

N_CORES = 8
B_FULL = 128
B_LOC = B_FULL // N_CORES  # 16
C = 32
H = 16
S = H * H  # 256
V = 4096
PNS = (1, 2, 4, 8)
ROW_OFF = (0, 4, 20, 84)
NTOK_OUT = 340

F32 = mybir.dt.float32
BF16 = mybir.dt.bfloat16
F16 = mybir.dt.float16
U16 = mybir.dt.uint16
U32 = mybir.dt.uint32
AX = mybir.AxisListType
ALU = mybir.AluOpType
ACTF = mybir.ActivationFunctionType

LAST_RESULTS = None


def _keys_cubic(x, a=-0.5):
    x = np.abs(x)
    return np.where(
        x <= 1,
        (a + 2) * x**3 - (a + 3) * x**2 + 1,
        np.where(x < 2, a * x**3 - 5 * a * x**2 + 8 * a * x - 4 * a, 0.0),
    )


def _resize_matrix_1d(n_in, n_out):
    scale = n_out / n_in
    U = np.zeros((n_out, n_in), np.float64)
    for i in range(n_out):
        x = (i + 0.5) / scale - 0.5
        w = _keys_cubic(x - np.arange(n_in))
        s = w.sum()
        if s != 0:
            w = w / s
        U[i] = w
    return U


def _up_matrix(pn):
    if pn == H:
        return np.eye(S)
    U1 = _resize_matrix_1d(pn, H)
    return np.kron(U1, U1)


def _down_matrix(pn):
    r = H // pn
    A = np.zeros((pn * pn, S), np.float64)
    w = 1.0 / (r * r)
    for pi in range(pn):
        for pj in range(pn):
            for di in range(r):
                for dj in range(r):
                    A[pi * pn + pj, (pi * r + di) * H + (pj * r + dj)] = w
    return A


def _to_bf16(a):
    import ml_dtypes

    return np.ascontiguousarray(np.asarray(a).astype(np.float32).astype(ml_dtypes.bfloat16))


def _build_program():
    nc = bacc.Bacc(trn_type="TRN2", target_bir_lowering=False, debug=False)

    eaug_in = nc.dram_tensor("eaug", [97, V], F32, kind="ExternalInput").ap()
    embB_in = nc.dram_tensor("embB", [V // 2, 66], F32, kind="ExternalInput").ap()
    i33_in = nc.dram_tensor("i33", [33, 132], F32, kind="ExternalInput").ap()
    zaug0_in = nc.dram_tensor("zaug0", [97, 16], F32, kind="ExternalInput").ap()
    f_in = nc.dram_tensor("f_pre", [2, 128, 512], F32, kind="ExternalInput").ap()
    a_in = {
        si: nc.dram_tensor(
            f"a{si}", [2, 128, PNS[si] ** 2], F32, kind="ExternalInput"
        ).ap()
        for si in (1, 2, 3)
    }
    # negated correction mats, zero-padded to K=128: [128, P_si]
    mau_in = {
        (si, k): nc.dram_tensor(
            f"mau{si}_{k}", [128, PNS[si] ** 2], F32, kind="ExternalInput"
        ).ap()
        for si in (1, 2, 3)
        for k in range(si)
    }
    # x mats (bf16): gx[si][k] zero-padded to K=32: [32, P_{si+1}] for si<3
    gx_in = {
        (si, k): nc.dram_tensor(
            f"gx{si}_{k}", [32, PNS[si + 1] ** 2], BF16, kind="ExternalInput"
        ).ap()
        for si in range(3)
        for k in range(si + 1)
    }
    # x3 mats zero-padded to K=64: [64, 256]
    ux_in = {
        k: nc.dram_tensor(f"ux{k}", [64, 256], BF16, kind="ExternalInput").ap()
        for k in range(4)
    }
    x_out = nc.dram_tensor("xout", [NTOK_OUT, 512], F32, kind="ExternalOutput").ap()

    with tile.TileContext(nc) as tc:
        from contextlib import ExitStack

        ctx = ExitStack()
        const = ctx.enter_context(tc.tile_pool(name="const", bufs=1))
        work = ctx.enter_context(tc.tile_pool(name="work", bufs=2))
        small = ctx.enter_context(tc.tile_pool(name="small", bufs=3))
        psA = ctx.enter_context(tc.tile_pool(name="psA", bufs=2, space="PSUM"))
        psB = ctx.enter_context(tc.tile_pool(name="psB", bufs=1, space="PSUM"))

        # ---- constants ----
        eaug = const.tile([97, V], F32)
        nc.sync.dma_start(eaug[0:33], eaug_in[0:33])
        nc.scalar.dma_start(eaug[64:97], eaug_in[64:97])
        f_pre = [const.tile([128, 512], F32, name=f"fpre{ch}") for ch in range(2)]
        for ch in range(2):
            nc.sync.dma_start(f_pre[ch], f_in[ch])
        zaug = {0: const.tile([97, 16], F32, name="zaug0")}
        nc.scalar.dma_start(zaug[0], zaug0_in)
        i33 = const.tile([33, 132], F32, name="i33")
        nc.scalar.dma_start(i33, i33_in)
        for si in (1, 2, 3):
            P = PNS[si] ** 2
            zaug[si] = const.tile([97, 16 * P], F32, name=f"zaug{si}")
            nc.vector.memset(zaug[si][32:33], 1.0)
            nc.vector.memset(zaug[si][96:97], 1.0)
        a_sb = {}
        for si in (1, 2, 3):
            P = PNS[si] ** 2
            a_sb[si] = [const.tile([128, P], F32, name=f"a{si}_{ch}") for ch in range(2)]
            for ch in range(2):
                nc.sync.dma_start(a_sb[si][ch], a_in[si][ch])
        mau = {}
        for key, ap in mau_in.items():
            mau[key] = const.tile(list(ap.shape), F32, name=f"mau{key[0]}_{key[1]}")
            nc.scalar.dma_start(mau[key], ap)
        gx = {}
        for key, ap in gx_in.items():
            gx[key] = const.tile(list(ap.shape), BF16, name=f"gx{key[0]}_{key[1]}")
            nc.sync.dma_start(gx[key], ap)
        ux = {}
        for k, ap in ux_in.items():
            ux[k] = const.tile(list(ap.shape), BF16, name=f"ux{k}")
            nc.scalar.dma_start(ux[k], ap)

        # h state: fp32 padded to 128 rows (corrections, K=128);
        # bf16 padded to 64 rows (x matmuls, K=32/64)
        h_sp = {}
        h16 = {}
        for si in range(4):
            if si < 3:
                h_sp[si] = const.tile([128, 512], F32, name=f"h{si}")
                nc.vector.memset(h_sp[si], 0.0)
            else:
                h_sp[si] = const.tile([64, 512], F32, name="h3")
                nc.vector.memset(h_sp[si], 0.0)
            h16[si] = const.tile([64, 512], BF16, name=f"h16_{si}")
            nc.vector.memset(h16[si], 0.0)

        for si, pn in enumerate(PNS):
            P = pn * pn
            ntok = B_LOC * P
            nblk = (ntok + 127) // 128

            # ---- z = A f - corrections -> fold into zaug (si>0) ----
            if si > 0:
                for g in range(4):
                    psz = psB.tile([128, 64], F32, tag="psz")
                    gsl = slice(128 * g, 128 * (g + 1))
                    for ch in range(2):
                        nc.tensor.matmul(
                            psz[:, :P],
                            f_pre[ch][:, gsl],
                            a_sb[si][ch],
                            start=(ch == 0),
                            stop=False,
                        )
                    for k in range(si):
                        nc.tensor.matmul(
                            psz[:, :P],
                            h_sp[k][:, gsl],
                            mau[(si, k)],
                            start=False,
                            stop=(k == si - 1),
                        )
                    for j in range(4):
                        b = 4 * g + j
                        nc.scalar.activation(
                            zaug[si][0:32, b * P : (b + 1) * P],
                            psz[32 * j : 32 * (j + 1), :P],
                            ACTF.Copy,
                        )
                    nc.gpsimd.dma_start(
                        zaug[si][64:97, 4 * g * P : 4 * (g + 1) * P],
                        zaug[si][0:33, 4 * g * P : 4 * (g + 1) * P],
                    )

            # ---- token blocks ----
            for blk in range(nblk):
                t0 = blk * 128
                tw = min(128, ntok - t0)
                zb = zaug[si][:, t0 : t0 + tw]

                sb1 = work.tile([128, 2048], F32, tag="sb1")
                merged = work.tile([128, 2048], F16, tag="mrg")
                for h in range(2):
                    c0 = psA.tile([128, 1024], F32, tag="c0")
                    for q in range(2):
                        cs = slice(512 * q, 512 * (q + 1))
                        e0 = 1024 * h + 512 * q
                        nc.tensor.matmul(
                            c0[:tw, cs],
                            zb[0:33],
                            eaug[0:33, e0 : e0 + 512],
                            start=True,
                            stop=True,
                            tile_position=(0, 0),
                        )
                        c1 = psA.tile([128, 512], F32, tag="c1")
                        nc.tensor.matmul(
                            c1[:tw],
                            zb[64:97],
                            eaug[64:97, 2048 + e0 : 2048 + e0 + 512],
                            start=True,
                            stop=True,
                            tile_position=(64, 0),
                        )
                        nc.scalar.activation(
                            sb1[:tw, 1024 * h + cs.start : 1024 * h + cs.stop],
                            c1[:tw],
                            ACTF.Copy,
                        )
                    hs = slice(1024 * h, 1024 * (h + 1))
                    nc.vector.tensor_tensor(
                        merged[:tw, hs], c0[:tw], sb1[:tw, hs], op=ALU.max
                    )

                top8 = small.tile([128, 8], F16, tag="top8")
                j8 = small.tile([128, 8], U32, tag="j8")
                nc.vector.max(top8[:tw], merged[:tw])
                nc.vector.max_index(j8[:tw], top8[:tw], merged[:tw])

                # zrow4 via PE transpose with 4-replicated identity
                zrp = psB.tile([128, 512], F32, tag="px")
                nc.tensor.transpose(zrp[:tw, 0:132], zb[0:33], i33)
                zrow4 = small.tile([128, 132], F32, tag="zrow4")
                nc.scalar.activation(zrow4[:tw], zrp[:tw, 0:132], ACTF.Copy)

                # 4 candidates: pairs (j0, j0+2048), (j1, j1+2048); the
                # repacked embB2 row j holds both chunk embeddings adjacently
                e4 = small.tile([128, 132], F32, tag="e4")
                nc.gpsimd.indirect_dma_start(
                    e4[:tw, 0:66], None, embB_in,
                    IndirectOffsetOnAxis(ap=j8[:tw, 0:1], axis=0),
                )
                nc.gpsimd.indirect_dma_start(
                    e4[:tw, 66:132], None, embB_in,
                    IndirectOffsetOnAxis(ap=j8[:tw, 1:2], axis=0),
                )
                # rescore all 4 candidates exactly
                t4 = small.tile([128, 132], F32, tag="t4")
                nc.vector.tensor_tensor(t4[:tw], zrow4[:tw], e4[:tw], op=ALU.mult)
                s4 = small.tile([128, 4], F32, tag="s4")
                nc.vector.reduce_sum(
                    s4[:tw], t4[:tw].rearrange("t (k c) -> t k c", k=4), axis=AX.X
                )
                # tournament select of embedding rows (scan-order tie-safe:
                # strict greater-than prefers the earlier candidate)
                g01 = small.tile([128, 1], F32, tag="g01")
                g23 = small.tile([128, 1], F32, tag="g23")
                nc.vector.tensor_tensor(g01[:tw], s4[:tw, 0:1], s4[:tw, 1:2], op=ALU.is_lt)
                nc.vector.tensor_tensor(g23[:tw], s4[:tw, 2:3], s4[:tw, 3:4], op=ALU.is_lt)
                d01 = small.tile([128, C], F32, tag="d01")
                d23 = small.tile([128, C], F32, tag="d23")
                nc.vector.tensor_tensor(
                    d01[:tw], e4[:tw, 33:65], e4[:tw, 0:32], op=ALU.subtract
                )
                nc.vector.tensor_tensor(
                    d23[:tw], e4[:tw, 99:131], e4[:tw, 66:98], op=ALU.subtract
                )
                e01 = small.tile([128, C], F32, tag="e01")
                e23 = small.tile([128, C], F32, tag="e23")
                nc.vector.scalar_tensor_tensor(
                    e01[:tw], in0=d01[:tw], scalar=g01[:tw], in1=e4[:tw, 0:32],
                    op0=ALU.mult, op1=ALU.add,
                )
                nc.vector.scalar_tensor_tensor(
                    e23[:tw], in0=d23[:tw], scalar=g23[:tw], in1=e4[:tw, 66:98],
                    op0=ALU.mult, op1=ALU.add,
                )
                s01 = small.tile([128, 1], F32, tag="s01")
                s23 = small.tile([128, 1], F32, tag="s23")
                nc.vector.tensor_tensor(s01[:tw], s4[:tw, 0:1], s4[:tw, 1:2], op=ALU.max)
                nc.vector.tensor_tensor(s23[:tw], s4[:tw, 2:3], s4[:tw, 3:4], op=ALU.max)
                gf = small.tile([128, 1], F32, tag="gf")
                nc.vector.tensor_tensor(gf[:tw], s01[:tw], s23[:tw], op=ALU.is_lt)
                df = small.tile([128, C], F32, tag="df")
                nc.vector.tensor_tensor(df[:tw], e23[:tw], e01[:tw], op=ALU.subtract)
                htok = small.tile([128, C], F32, tag="htok")
                nc.vector.scalar_tensor_tensor(
                    htok[:tw], in0=df[:tw], scalar=gf[:tw], in1=e01[:tw],
                    op0=ALU.mult, op1=ALU.add,
                )
                # scatter into h_sp [p, (b,c)], one DMA per image
                qs = [nc.sync, nc.scalar, nc.gpsimd]
                for j, b in enumerate(range(t0 // P, (t0 + tw + P - 1) // P)):
                    r0 = b * P - t0
                    qs[j % 3].dma_start(
                        h_sp[si][0:P, 32 * b : 32 * (b + 1)],
                        htok[r0 : r0 + P],
                    )

            # ---- h16 convert ----
            nc.scalar.activation(h16[si][0:P], h_sp[si][0:P], ACTF.Copy)

            # ---- x output ----
            if si < 3:
                P2 = PNS[si + 1] ** 2
                px = psB.tile([128, 512], F32, tag="px")
                for k in range(si + 1):
                    nc.tensor.matmul(
                        px[:P2],
                        gx[(si, k)],
                        h16[k][0:32],
                        start=(k == 0),
                        stop=(k == si),
                    )
                x_sb = small.tile([128, 512], F32, tag="xsb")
                nc.scalar.activation(x_sb[:P2], px[:P2], ACTF.Copy)
                nc.sync.dma_start(x_out[ROW_OFF[si] : ROW_OFF[si] + P2], x_sb[:P2])
            else:
                for ch in range(2):
                    px = psB.tile([128, 512], F32, tag="px")
                    for k in range(4):
                        nc.tensor.matmul(
                            px,
                            ux[k][:, 128 * ch : 128 * (ch + 1)],
                            h16[k],
                            start=(k == 0),
                            stop=(k == 3),
                        )
                    x_sb = small.tile([128, 512], F32, tag="xsb")
                    nc.scalar.activation(x_sb, px, ACTF.Copy)
                    nc.sync.dma_start(
                        x_out[84 + 128 * ch : 84 + 128 * (ch + 1)], x_sb
                    )

        ctx.close()

    nc.compile()
    return nc


_PROGRAM = None


def _get_program():
    global _PROGRAM
    if _PROGRAM is None:
        _PROGRAM = _build_program()
    return _PROGRAM


def _host_prep(f_src, emb_weight):
    e64 = emb_weight.astype(np.float64)
    bias = -0.5 * (e64 * e64).sum(1)
    eaug = np.zeros((97, V), np.float64)
    eaug[0:32] = e64.T
    eaug[32] = bias
    eaug[64:96] = e64.T
    eaug[96] = bias
    eaug = eaug.astype(np.float32)
    embB1 = np.concatenate([e64, bias[:, None]], axis=1)  # [V, 33]
    embB = np.ascontiguousarray(
        np.concatenate([embB1[: V // 2], embB1[V // 2 :]], axis=1).astype(np.float32)
    )  # [V/2, 66]: row j = [aug(e_j) | aug(e_{j+2048})]

    A = {si: _down_matrix(pn) for si, pn in enumerate(PNS)}
    U = {si: _up_matrix(pn) for si, pn in enumerate(PNS)}

    a_mats = {
        si: np.ascontiguousarray(
            A[si].T.reshape(2, 128, PNS[si] ** 2).astype(np.float32)
        )
        for si in (1, 2, 3)
    }
    mau = {}
    for si in (1, 2, 3):
        for k in range(si):
            m = A[si] @ U[k]  # [P_si, P_k]
            pad = np.zeros((128, PNS[si] ** 2), np.float64)
            pad[0 : PNS[k] ** 2] = -m.T
            mau[(si, k)] = pad.astype(np.float32)
    gx = {}
    for si in range(3):
        for k in range(si + 1):
            m = A[si + 1] @ U[k]
            pad = np.zeros((32, PNS[si + 1] ** 2))
            pad[0 : PNS[k] ** 2] = m.T
            gx[(si, k)] = _to_bf16(pad)
    ux = {}
    for k in range(4):
        pad = np.zeros((64, 256))
        pad[0 : PNS[k] ** 2] = U[k].T
        ux[k] = _to_bf16(pad)

    f64 = f_src.astype(np.float64).reshape(B_FULL, C, S)

    in_maps = []
    for core in range(N_CORES):
        fb = f64[core * B_LOC : (core + 1) * B_LOC]  # [16, 32, 256]
        f_pre = np.ascontiguousarray(
            fb.transpose(2, 0, 1).reshape(2, 128, 512).astype(np.float32)
        )  # [s-chunk, s, (b,c)]
        z0 = fb.mean(axis=2)  # [16, 32]
        za0 = np.zeros((97, 16), np.float64)
        za0[0:32] = z0.T
        za0[32] = 1.0
        za0[64:96] = z0.T
        za0[96] = 1.0
        m = {
            "eaug": eaug,
            "embB": embB,
            "i33": np.ascontiguousarray(np.tile(np.eye(33, dtype=np.float32), (1, 4))),
            "zaug0": za0.astype(np.float32),
            "f_pre": f_pre,
        }
        for si in (1, 2, 3):
            m[f"a{si}"] = a_mats[si]
        for (si, k), v in mau.items():
            m[f"mau{si}_{k}"] = v
        for (si, k), v in gx.items():
            m[f"gx{si}_{k}"] = v
        for k, v in ux.items():
            m[f"ux{k}"] = v
        in_maps.append(m)
    return in_maps


def kernel(f_src, emb_weight):
    global LAST_RESULTS
    f_src = np.asarray(f_src, dtype=np.float32)
    emb_weight = np.asarray(emb_weight, dtype=np.float32)

    in_maps = _host_prep(f_src, emb_weight)

    nc = _get_program()
    trace = bool(os.environ.get("CVAR_TRACE"))
    try:
        res = run_bass_kernel_spmd(
            nc, in_maps, core_ids=list(range(N_CORES)), trace=trace
        )
    except ModuleNotFoundError:
        res = run_bass_kernel_spmd(
            nc, in_maps, core_ids=list(range(N_CORES)), trace=False
        )
    LAST_RESULTS = res

    outs = []
    for core in range(N_CORES):
        xo = res.results[core]["xout"]  # [340, 512] in [p', (b,c)]
        outs.append(xo.reshape(NTOK_OUT, B_LOC, C).transpose(1, 0, 2))
    return np.ascontiguousarray(np.concatenate(outs, axis=0))
